# revision 9
# baseline (speedup 1.0000x reference)
"""BDGCN (dual-diffusion graph conv) Trainium2 kernel.

Math (per batch b):
  m1[k,m,c,l] = sum_n X[n,c,l] G[k,n,m]
  m2[m,d,k,j,l] = sum_c m1[k,m,c,l] G[j,c,d]
  out[m,d,h] = relu(sum_{k,j,l} m2[m,d,k,j,l] W[k*96+j*32+l, h] + b[h])

Sharding: data-parallel over batch; B=8 -> one batch per NeuronCore,
G/W/b replicated. No collectives.

Per-core pipeline (all fp32):
  Phase 1: lhsT = X[:, c-chunk, l] [n,128c], rhs = G_k [n, m-half-chunk]
           -> psum [c128, m128], accum over 2 n-chunks
           -> M1[k][cchk] SBUF [c128, (l32, m128)]
  Phase 2: lhsT = M1 view [c128, (m4, l32)cols], rhs = G_j [c128, d256]
           -> psum [(m4,l32)128, d256], accum over 2 c-chunks
           -> M2 SBUF tiles per (k,j)
  Phase 3: per m' (4, row-tiled at partition base 32*m'):
           lhsT = M2[kj][32m':+32, d-chunk], rhs = Wr[kj][32m':+32, :64]
           -> psum [d128, h64], accum over 9 (k,j)
           -> +bias (DVE), relu (ACT) -> out staging -> DMA [m,d,h]
"""

import numpy as np

B, N, L, K, H = 8, 256, 32, 3, 64
P = 128  # partitions

_CACHE = {}


def _patch_tile_drain():
    """This container's walrus build rejects instructions carrying more
    than one semaphore wait; Tile's exit emits one drain with N waits.
    Split it into N single-wait drains."""
    import concourse.mybir as mybir
    import concourse.tile as tile

    if getattr(tile.TileContext, "_drain_split_patched", False):
        return

    def patched(self, tick_clock, wait_clock):
        from concourse.vector_clock import ScopedClock

        nc = self.nc
        probe = nc.sync.drain()
        wait_clock.add_sem_waits(
            probe.ins, ScopedClock({None: tick_clock.global_clock})
        )
        si = probe.ins.sync_info
        waits = list(si.on_wait) if si is not None else []
        if len(waits) > 1:
            si.on_wait = [waits[0]]
            for w in waits[1:]:
                d = nc.sync.drain()
                d.ins.sync_info = mybir.SyncInfo(on_update=[], on_wait=[w])
        nc.all_engine_barrier()
        assert self.sems is not None
        popped = nc._tile_sem_poison_stack.pop()
        assert popped is self._sem_poison
        nc.clear_and_free_semaphores(list(self.sems.allocated().values()))
        nc.all_engine_barrier()

    tile.TileContext._drain_and_barrier = patched
    tile.TileContext._drain_split_patched = True


def _build_nc():
    import concourse.bass as bass
    import concourse.mybir as mybir
    import concourse.tile as tile
    from concourse import bacc

    _patch_tile_drain()

    f32 = mybir.dt.float32
    nc = bass.Bass("TRN2", target_bir_lowering=False, debug=False)

    Xd = nc.dram_tensor("X", [N, N, L], f32, kind="ExternalInput")
    Gd = nc.dram_tensor("G", [K, N, N], f32, kind="ExternalInput")
    Wr = nc.dram_tensor("WR", [K * K, P, H], f32, kind="ExternalInput")
    Bd = nc.dram_tensor("BB", [P, H], f32, kind="ExternalInput")
    Od = nc.dram_tensor("OUT", [N, N, H], f32, kind="ExternalOutput")

    NC2 = N // P  # 2 chunks of 128 along n or c
    MG = 4       # m's per group in phase 2/3
    NG = P // MG  # 32 groups per m-half

    with tile.TileContext(nc) as tc:
        with (
            tc.tile_pool(name="big", bufs=1) as big,
            tc.tile_pool(name="m2p", bufs=12) as m2p,
            tc.tile_pool(name="outp", bufs=4) as outp,
            tc.tile_pool(name="ps1", bufs=2, space="PSUM") as ps1p,
            tc.tile_pool(name="ps2", bufs=2, space="PSUM") as ps2p,
            tc.tile_pool(name="ps3", bufs=4, space="PSUM") as ps3p,
        ):
            # ---- resident loads ----
            xsb = big.tile([P, NC2 * N * L], f32, tag="xsb")
            x4 = xsb.rearrange("p (b c l) -> p b c l", b=NC2, c=N)
            nc.sync.dma_start(
                out=x4, in_=Xd[:, :, :].rearrange("(b p) c l -> p b c l", p=P)
            )
            gsb = big.tile([P, K * NC2 * N], f32, tag="gsb")
            g4 = gsb.rearrange("p (k b m) -> p k b m", k=K, b=NC2)
            nc.sync.dma_start(
                out=g4, in_=Gd[:, :, :].rearrange("k (b p) m -> p k b m", p=P)
            )
            wsb = big.tile([P, K * K * H], f32, tag="wsb")
            w3 = wsb.rearrange("p (q h) -> p q h", q=K * K)
            nc.sync.dma_start(out=w3, in_=Wr[:, :, :].rearrange("q p h -> p q h"))
            bsb = big.tile([P, H], f32, tag="bsb")
            nc.sync.dma_start(out=bsb, in_=Bd[:, :])

            m1 = {}
            for k in range(K):
                for cc in range(NC2):
                    m1t = big.tile([P, L * P], f32, tag=f"m1_{k}_{cc}", name=f"m1_{k}_{cc}")
                    m1[k, cc] = m1t

            for mh in range(2):  # m-half
                # ---- phase 1 ----
                for k in range(K):
                    for cc in range(NC2):
                        m1w = m1[k, cc].rearrange("p (m l) -> p m l", l=L)
                        for l in range(L):
                            ps = ps1p.tile([P, P], f32, tag="ps1")
                            for nchk in range(NC2):
                                nc.tensor.matmul(
                                    ps,
                                    lhsT=x4[:, nchk, cc * P:(cc + 1) * P, l],
                                    rhs=g4[:, k, nchk, mh * P:(mh + 1) * P],
                                    start=(nchk == 0),
                                    stop=(nchk == NC2 - 1),
                                )
                            nc.vector.tensor_copy(m1w[:, :, l], ps)
                # ---- phases 2 + 3, per group of 4 m's ----
                for g in range(NG):
                    m2sb = {}
                    for k in range(K):
                        for j in range(K):
                            ps2 = ps2p.tile([P, N], f32, tag="ps2")
                            for cc in range(NC2):
                                lv = m1[k, cc][:, g * P:(g + 1) * P]
                                nc.tensor.matmul(
                                    ps2,
                                    lhsT=lv,
                                    rhs=g4[:, j, cc, :],
                                    start=(cc == 0),
                                    stop=(cc == NC2 - 1),
                                )
                            t = m2p.tile([P, N], f32, tag="m2")
                            nc.vector.tensor_copy(t, ps2)
                            m2sb[k, j] = t
                    for dc in range(NC2):
                        pss = [ps3p.tile([P, H], f32, tag="ps3", name=f"ps3_{mp}") for mp in range(MG)]
                        for idx in range(K * K):
                            k, j = divmod(idx, K)
                            for mp in range(MG):
                                r0 = 32 * mp
                                nc.tensor.matmul(
                                    pss[mp],
                                    lhsT=m2sb[k, j][r0:r0 + 32, dc * P:(dc + 1) * P],
                                    rhs=w3[r0:r0 + 32, idx, :],
                                    start=(idx == 0),
                                    stop=(idx == K * K - 1),
                                    tile_position=(r0, 0),
                                )
                        ost = outp.tile([P, MG * H], f32, tag="ost")
                        for mp in range(MG):
                            sl = ost[:, mp * H:(mp + 1) * H]
                            nc.vector.scalar_tensor_tensor(
                                out=sl,
                                in0=pss[mp],
                                scalar=0.0,
                                in1=bsb,
                                op0=mybir.AluOpType.add,
                                op1=mybir.AluOpType.add,
                            )
                            nc.gpsimd.tensor_scalar_max(sl, sl, 0.0)
                        mbase = mh * P + g * MG
                        dst = Od[mbase:mbase + MG, dc * P:(dc + 1) * P, :]
                        nc.sync.dma_start(
                            out=dst.rearrange("m d h -> d m h"), in_=ost
                        )
    _split_multi_waits(nc)
    return nc


def _split_multi_waits(nc):
    """This walrus build accepts at most one semaphore wait per
    instruction; Tile emits up to ~2-4.  Hoist extra waits onto NoOp
    instructions inserted just before, on the same engine."""
    import concourse.mybir as mybir

    n_split = 0
    for fn in nc.m.functions:
        for bb in fn.blocks:
            insts = bb.instructions
            new = []
            for inst in insts:
                si = inst.sync_info
                waits = list(si.on_wait) if si is not None else []
                if len(waits) > 1:
                    for w in waits[:-1]:
                        nop = mybir.InstNoOp(
                            name=nc.get_next_instruction_name(), ins=[], outs=[]
                        )
                        nop.engine = inst.engine
                        nop.sync_info = mybir.SyncInfo(
                            on_update=[], on_wait=[w]
                        )
                        new.append(nop)
                        n_split += 1
                    si.on_wait = [waits[-1]]
                new.append(inst)
            if n_split:
                bb.instructions = new
    return n_split


def _get_nc():
    if "nc" not in _CACHE:
        _CACHE["nc"] = _build_nc()
    return _CACHE["nc"]


def _prep(G, W, b):
    # Wr[k*3+j] = tile of W rows [k*96+j*32 : +32], replicated 4x along
    # partitions to line up with the (m4, l32) psum layout of phase 2.
    Wr = np.empty((K * K, P, H), dtype=np.float32)
    for k in range(K):
        for j in range(K):
            blk = W[k * (K * L) + j * L:k * (K * L) + (j + 1) * L, :]
            Wr[k * K + j] = np.tile(blk, (4, 1))
    Bb = np.tile(b[None, :], (P, 1)).astype(np.float32)
    return np.ascontiguousarray(Wr), Bb


def kernel(X, G, W, b):
    from concourse.bass_utils import run_bass_kernel_spmd

    X = np.ascontiguousarray(X, dtype=np.float32)
    G = np.ascontiguousarray(G, dtype=np.float32)
    W = np.ascontiguousarray(W, dtype=np.float32)
    b = np.ascontiguousarray(b, dtype=np.float32)
    nc = _get_nc()
    Wr, Bb = _prep(G, W, b)
    in_maps = [
        {"X": X[i], "G": G, "WR": Wr, "BB": Bb} for i in range(B)
    ]
    res = run_bass_kernel_spmd(nc, in_maps, list(range(B)))
    out = np.stack([res.results[i]["OUT"] for i in range(B)], axis=0)
    return out


# revision 12
# speedup vs baseline: 4.5667x; 4.5667x over previous
"""BDGCN (dual-diffusion graph conv) Trainium2 kernel.

Math (per batch b):
  m1[k,m,c,l] = sum_n X[n,c,l] G[k,n,m]
  m2[m,d,k,j,l] = sum_c m1[k,m,c,l] G[j,c,d]
  out[m,d,h] = relu(sum_{k,j,l} m2[m,d,k,j,l] W[k*96+j*32+l, h] + b[h])

Sharding: data-parallel over batch; B=8 -> one batch per NeuronCore,
G/W/b replicated. No collectives.

Per-core pipeline (all fp32):
  Phase 1: lhsT = X[:, c-chunk, l] [n,128c], rhs = G_k [n, m-half-chunk]
           -> psum [c128, m128], accum over 2 n-chunks
           -> M1[k][cchk] SBUF [c128, (l32, m128)]
  Phase 2: lhsT = M1 view [c128, (m4, l32)cols], rhs = G_j [c128, d256]
           -> psum [(m4,l32)128, d256], accum over 2 c-chunks
           -> M2 SBUF tiles per (k,j)
  Phase 3: per m' (4, row-tiled at partition base 32*m'):
           lhsT = M2[kj][32m':+32, d-chunk], rhs = Wr[kj][32m':+32, :64]
           -> psum [d128, h64], accum over 9 (k,j)
           -> +bias (DVE), relu (ACT) -> out staging -> DMA [m,d,h]
"""

import numpy as np

B, N, L, K, H = 8, 256, 32, 3, 64
P = 128  # partitions

_CACHE = {}


def _patch_tile_drain():
    """This container's walrus build rejects instructions carrying more
    than one semaphore wait; Tile's exit emits one drain with N waits.
    Split it into N single-wait drains."""
    import concourse.mybir as mybir
    import concourse.tile as tile

    if getattr(tile.TileContext, "_drain_split_patched", False):
        return

    def patched(self, tick_clock, wait_clock):
        from concourse.vector_clock import ScopedClock

        nc = self.nc
        probe = nc.sync.drain()
        wait_clock.add_sem_waits(
            probe.ins, ScopedClock({None: tick_clock.global_clock})
        )
        si = probe.ins.sync_info
        waits = list(si.on_wait) if si is not None else []
        if len(waits) > 1:
            si.on_wait = [waits[0]]
            for w in waits[1:]:
                d = nc.sync.drain()
                d.ins.sync_info = mybir.SyncInfo(on_update=[], on_wait=[w])
        nc.all_engine_barrier()
        assert self.sems is not None
        popped = nc._tile_sem_poison_stack.pop()
        assert popped is self._sem_poison
        nc.clear_and_free_semaphores(list(self.sems.allocated().values()))
        nc.all_engine_barrier()

    tile.TileContext._drain_and_barrier = patched
    tile.TileContext._drain_split_patched = True


def _build_nc(reps=1):
    import concourse.bass as bass
    import concourse.mybir as mybir
    import concourse.tile as tile
    from concourse import bacc

    _patch_tile_drain()

    f32 = mybir.dt.float32
    nc = bass.Bass("TRN2", target_bir_lowering=False, debug=False)

    Xd = nc.dram_tensor("X", [N, N, L], f32, kind="ExternalInput")
    Gd = nc.dram_tensor("G", [K, N, N], f32, kind="ExternalInput")
    Wr = nc.dram_tensor("WR", [K * K, P, 4 * H], f32, kind="ExternalInput")
    Bd = nc.dram_tensor("BB", [P, 4 * H], f32, kind="ExternalInput")
    Od = nc.dram_tensor("OUT", [N, N, H], f32, kind="ExternalOutput")

    NC2 = N // P  # 2 chunks of 128 along n or c
    MG = 4       # m's per group in phase 2/3
    NG = P // MG  # 32 groups per m-half

    with tile.TileContext(nc) as tc:
        with (
            tc.tile_pool(name="big", bufs=1) as big,
            tc.tile_pool(name="m2p", bufs=12) as m2p,
            tc.tile_pool(name="outp", bufs=4) as outp,
            tc.tile_pool(name="ps1", bufs=2, space="PSUM") as ps1p,
            tc.tile_pool(name="ps2", bufs=2, space="PSUM") as ps2p,
            tc.tile_pool(name="ps3", bufs=4, space="PSUM") as ps3p,
        ):
            # ---- resident loads ----
            xsb = big.tile([P, NC2 * N * L], f32, tag="xsb")
            x4 = xsb.rearrange("p (b c l) -> p b c l", b=NC2, c=N)
            nc.sync.dma_start(
                out=x4, in_=Xd[:, :, :].rearrange("(b p) c l -> p b c l", p=P)
            )
            gsb = big.tile([P, K * NC2 * N], f32, tag="gsb")
            g4 = gsb.rearrange("p (k b m) -> p k b m", k=K, b=NC2)
            nc.sync.dma_start(
                out=g4, in_=Gd[:, :, :].rearrange("k (b p) m -> p k b m", p=P)
            )
            wsb = big.tile([P, K * K * MG * H], f32, tag="wsb")
            w3 = wsb.rearrange("p (q c) -> p q c", q=K * K)
            nc.sync.dma_start(out=w3, in_=Wr[:, :, :].rearrange("q p c -> p q c"))
            bsb = big.tile([P, MG * H], f32, tag="bsb")
            nc.sync.dma_start(out=bsb, in_=Bd[:, :])

            m1 = {}
            for k in range(K):
                for cc in range(NC2):
                    m1t = big.tile([P, L * P], f32, tag=f"m1_{k}_{cc}", name=f"m1_{k}_{cc}")
                    m1[k, cc] = m1t

            for _rep in range(reps):
              for mh in range(2):  # m-half
                # ---- phase 1 ----
                for k in range(K):
                    for cc in range(NC2):
                        # M1 free layout: (g32, l32, r4) with m = g*4 + r
                        m1w = m1[k, cc].rearrange(
                            "p (g l r) -> p g l r", g=NG, l=L
                        )
                        for l in range(L):
                            ps = ps1p.tile([P, P], f32, tag="ps1")
                            for nchk in range(NC2):
                                nc.tensor.matmul(
                                    ps,
                                    lhsT=x4[:, nchk, cc * P:(cc + 1) * P, l],
                                    rhs=g4[:, k, nchk, mh * P:(mh + 1) * P],
                                    start=(nchk == 0),
                                    stop=(nchk == NC2 - 1),
                                )
                            nc.vector.tensor_copy(m1w[:, :, l, :], ps)
                # ---- phases 2 + 3, per group of 4 m's ----
                for g in range(NG):
                    m2sb = {}
                    for k in range(K):
                        for j in range(K):
                            ps2 = ps2p.tile([P, N], f32, tag="ps2")
                            for cc in range(NC2):
                                lv = m1[k, cc][:, g * P:(g + 1) * P]
                                nc.tensor.matmul(
                                    ps2,
                                    lhsT=lv,
                                    rhs=g4[:, j, cc, :],
                                    start=(cc == 0),
                                    stop=(cc == NC2 - 1),
                                )
                            t = m2p.tile([P, N], f32, tag="m2")
                            nc.vector.tensor_copy(t, ps2)
                            m2sb[k, j] = t
                    for dc in range(NC2):
                        ps3 = ps3p.tile([P, MG * H], f32, tag="ps3")
                        for idx in range(K * K):
                            k, j = divmod(idx, K)
                            nc.tensor.matmul(
                                ps3,
                                lhsT=m2sb[k, j][:, dc * P:(dc + 1) * P],
                                rhs=w3[:, idx, :],
                                start=(idx == 0),
                                stop=(idx == K * K - 1),
                            )
                        ost = outp.tile([P, MG * H], f32, tag="ost")
                        nc.vector.scalar_tensor_tensor(
                            out=ost,
                            in0=ps3,
                            scalar=0.0,
                            in1=bsb,
                            op0=mybir.AluOpType.add,
                            op1=mybir.AluOpType.add,
                        )
                        nc.gpsimd.tensor_scalar_max(ost, ost, 0.0)
                        mbase = mh * P + g * MG
                        dst = Od[mbase:mbase + MG, dc * P:(dc + 1) * P, :]
                        nc.sync.dma_start(
                            out=dst.rearrange("m d h -> d m h"), in_=ost
                        )
    _split_multi_waits(nc)
    return nc


def _split_multi_waits(nc):
    """This walrus build accepts at most one semaphore wait per
    instruction; Tile emits up to ~2-4.  Hoist extra waits onto NoOp
    instructions inserted just before, on the same engine."""
    import concourse.mybir as mybir

    n_split = 0
    for fn in nc.m.functions:
        for bb in fn.blocks:
            insts = bb.instructions
            new = []
            for inst in insts:
                si = inst.sync_info
                waits = list(si.on_wait) if si is not None else []
                if len(waits) > 1:
                    for w in waits[:-1]:
                        nop = mybir.InstNoOp(
                            name=nc.get_next_instruction_name(), ins=[], outs=[]
                        )
                        nop.engine = inst.engine
                        nop.sync_info = mybir.SyncInfo(
                            on_update=[], on_wait=[w]
                        )
                        new.append(nop)
                        n_split += 1
                    si.on_wait = [waits[-1]]
                new.append(inst)
            if n_split:
                bb.instructions = new
    return n_split


def _get_nc():
    if "nc" not in _CACHE:
        _CACHE["nc"] = _build_nc()
    return _CACHE["nc"]


def _prep(G, W, b):
    # Block-diagonal W for phase 3: rows indexed (l, r) with r = m-within-
    # group, cols (r'', h); nonzero only when r == r''.
    MG = 4
    Wbd = np.zeros((K * K, P, MG * H), dtype=np.float32)
    for k in range(K):
        for j in range(K):
            blk = W[k * (K * L) + j * L:k * (K * L) + (j + 1) * L, :]  # [L, H]
            for l in range(L):
                for r in range(MG):
                    Wbd[k * K + j, l * MG + r, r * H:(r + 1) * H] = blk[l]
    Bb = np.tile(b[None, :], (P, MG)).astype(np.float32)
    return np.ascontiguousarray(Wbd), Bb


def kernel(X, G, W, b):
    from concourse.bass_utils import run_bass_kernel_spmd

    X = np.ascontiguousarray(X, dtype=np.float32)
    G = np.ascontiguousarray(G, dtype=np.float32)
    W = np.ascontiguousarray(W, dtype=np.float32)
    b = np.ascontiguousarray(b, dtype=np.float32)
    nc = _get_nc()
    Wr, Bb = _prep(G, W, b)
    in_maps = [
        {"X": X[i], "G": G, "WR": Wr, "BB": Bb} for i in range(B)
    ]
    res = run_bass_kernel_spmd(nc, in_maps, list(range(B)))
    out = np.stack([res.results[i]["OUT"] for i in range(B)], axis=0)
    return out


# revision 14
# speedup vs baseline: 43.5139x; 9.5285x over previous
"""BDGCN (dual-diffusion graph conv) Trainium2 kernel.

Math (per batch b):
  m1[k,m,c,l] = sum_n X[n,c,l] G[k,n,m]
  m2[m,d,k,j,l] = sum_c m1[k,m,c,l] G[j,c,d]
  out[m,d,h] = relu(sum_{k,j,l} m2[m,d,k,j,l] W[k*96+j*32+l, h] + b[h])

Sharding: data-parallel over batch; B=8 -> one batch per NeuronCore,
G/W/b replicated. No collectives.

Per-core pipeline (all fp32):
  Phase 1: lhsT = X[:, c-chunk, l] [n,128c], rhs = G_k [n, m-half-chunk]
           -> psum [c128, m128], accum over 2 n-chunks
           -> M1[k][cchk] SBUF [c128, (l32, m128)]
  Phase 2: lhsT = M1 view [c128, (m4, l32)cols], rhs = G_j [c128, d256]
           -> psum [(m4,l32)128, d256], accum over 2 c-chunks
           -> M2 SBUF tiles per (k,j)
  Phase 3: per m' (4, row-tiled at partition base 32*m'):
           lhsT = M2[kj][32m':+32, d-chunk], rhs = Wr[kj][32m':+32, :64]
           -> psum [d128, h64], accum over 9 (k,j)
           -> +bias (DVE), relu (ACT) -> out staging -> DMA [m,d,h]
"""

import numpy as np

B, N, L, K, H = 8, 256, 32, 3, 64
P = 128  # partitions

_CACHE = {}


def _patch_tile_drain():
    """This container's walrus build rejects instructions carrying more
    than one semaphore wait; Tile's exit emits one drain with N waits.
    Split it into N single-wait drains."""
    import concourse.mybir as mybir
    import concourse.tile as tile

    if getattr(tile.TileContext, "_drain_split_patched", False):
        return

    def patched(self, tick_clock, wait_clock):
        from concourse.vector_clock import ScopedClock

        nc = self.nc
        probe = nc.sync.drain()
        wait_clock.add_sem_waits(
            probe.ins, ScopedClock({None: tick_clock.global_clock})
        )
        si = probe.ins.sync_info
        waits = list(si.on_wait) if si is not None else []
        if len(waits) > 1:
            si.on_wait = [waits[0]]
            for w in waits[1:]:
                d = nc.sync.drain()
                d.ins.sync_info = mybir.SyncInfo(on_update=[], on_wait=[w])
        nc.all_engine_barrier()
        assert self.sems is not None
        popped = nc._tile_sem_poison_stack.pop()
        assert popped is self._sem_poison
        nc.clear_and_free_semaphores(list(self.sems.allocated().values()))
        nc.all_engine_barrier()

    tile.TileContext._drain_and_barrier = patched
    tile.TileContext._drain_split_patched = True


def _build_nc(reps=1):
    import concourse.bass as bass
    import concourse.mybir as mybir
    import concourse.tile as tile
    from concourse import bacc

    _patch_tile_drain()

    f32 = mybir.dt.float32
    f32r = mybir.dt.float32r
    nc = bass.Bass("TRN2", target_bir_lowering=False, debug=False)

    bf16 = mybir.dt.bfloat16
    Xd = nc.dram_tensor("X", [N, N, L], bf16, kind="ExternalInput")
    Gd = nc.dram_tensor("G", [K, N, N], f32, kind="ExternalInput")
    GBd = nc.dram_tensor("GB", [K, N, N], bf16, kind="ExternalInput")
    Wr = nc.dram_tensor("WR", [K * K, P, 4 * H], f32, kind="ExternalInput")
    Bd = nc.dram_tensor("BB", [P, 4 * H], f32, kind="ExternalInput")
    Od = nc.dram_tensor("OUT", [N, N, H], f32, kind="ExternalOutput")

    NC2 = N // P  # 2 chunks of 128 along n or c
    MG = 4       # m's per group in phase 2/3
    NG = P // MG  # 32 groups per m-half

    with tile.TileContext(nc) as tc:
        with (
            tc.tile_pool(name="big", bufs=1) as big,
            tc.tile_pool(name="m2p", bufs=12) as m2p,
            tc.tile_pool(name="outp", bufs=4) as outp,
            tc.tile_pool(name="ps1", bufs=2, space="PSUM") as ps1p,
            tc.tile_pool(name="ps2", bufs=2, space="PSUM") as ps2p,
            tc.tile_pool(name="ps3", bufs=4, space="PSUM") as ps3p,
        ):
            # ---- resident loads ----
            xsb = big.tile([P, NC2 * N * L], bf16, tag="xsb")
            x4 = xsb.rearrange("p (b c l) -> p b c l", b=NC2, c=N)
            nc.sync.dma_start(
                out=x4, in_=Xd[:, :, :].rearrange("(b p) c l -> p b c l", p=P)
            )
            gsb = big.tile([P, K * NC2 * N], f32r, tag="gsb")
            g4 = gsb.rearrange("p (k b m) -> p k b m", k=K, b=NC2)
            nc.sync.dma_start(
                out=g4,
                in_=Gd[:, :, :].bitcast(f32r).rearrange(
                    "k (b p) m -> p k b m", p=P
                ),
            )
            gbsb = big.tile([P, K * NC2 * N], bf16, tag="gbsb")
            gb4 = gbsb.rearrange("p (k b m) -> p k b m", k=K, b=NC2)
            nc.sync.dma_start(
                out=gb4,
                in_=GBd[:, :, :].rearrange("k (b p) m -> p k b m", p=P),
            )
            wsb = big.tile([P, K * K * MG * H], f32r, tag="wsb")
            w3 = wsb.rearrange("p (q c) -> p q c", q=K * K)
            nc.sync.dma_start(
                out=w3,
                in_=Wr[:, :, :].bitcast(f32r).rearrange("q p c -> p q c"),
            )
            bsb = big.tile([P, MG * H], f32, tag="bsb")
            nc.sync.dma_start(out=bsb, in_=Bd[:, :])

            m1 = {}
            for k in range(K):
                for cc in range(NC2):
                    m1t = big.tile([P, L * P], f32r, tag=f"m1_{k}_{cc}", name=f"m1_{k}_{cc}")
                    m1[k, cc] = m1t

            for _rep in range(reps):
              for mh in range(2):  # m-half
                # ---- phase 1 ----
                for k in range(K):
                    for cc in range(NC2):
                        # M1 free layout: (g32, l32, r4) with m = g*4 + r
                        m1w = m1[k, cc].rearrange(
                            "p (g l r) -> p g l r", g=NG, l=L
                        )
                        for l in range(L):
                            ps = ps1p.tile([P, P], f32, tag="ps1")
                            for nchk in range(NC2):
                                nc.tensor.matmul(
                                    ps,
                                    lhsT=x4[:, nchk, cc * P:(cc + 1) * P, l],
                                    rhs=gb4[:, k, nchk, mh * P:(mh + 1) * P],
                                    start=(nchk == 0),
                                    stop=(nchk == NC2 - 1),
                                )
                            nc.vector.tensor_copy(m1w[:, :, l, :], ps)
                # ---- phases 2 + 3, per group of 4 m's ----
                for g in range(NG):
                    m2sb = {}
                    for k in range(K):
                        for j in range(K):
                            ps2 = ps2p.tile([P, N], f32, tag="ps2")
                            for cc in range(NC2):
                                lv = m1[k, cc][:, g * P:(g + 1) * P]
                                nc.tensor.matmul(
                                    ps2,
                                    lhsT=lv,
                                    rhs=g4[:, j, cc, :],
                                    start=(cc == 0),
                                    stop=(cc == NC2 - 1),
                                )
                            t = m2p.tile([P, N], f32r, tag="m2")
                            nc.vector.tensor_copy(t, ps2)
                            m2sb[k, j] = t
                    for dc in range(NC2):
                        ps3 = ps3p.tile([P, MG * H], f32, tag="ps3")
                        for idx in range(K * K):
                            k, j = divmod(idx, K)
                            nc.tensor.matmul(
                                ps3,
                                lhsT=m2sb[k, j][:, dc * P:(dc + 1) * P],
                                rhs=w3[:, idx, :],
                                start=(idx == 0),
                                stop=(idx == K * K - 1),
                            )
                        ost = outp.tile([P, MG * H], f32, tag="ost")
                        nc.vector.scalar_tensor_tensor(
                            out=ost,
                            in0=ps3,
                            scalar=0.0,
                            in1=bsb,
                            op0=mybir.AluOpType.add,
                            op1=mybir.AluOpType.add,
                        )
                        nc.gpsimd.tensor_scalar_max(ost, ost, 0.0)
                        mbase = mh * P + g * MG
                        dst = Od[mbase:mbase + MG, dc * P:(dc + 1) * P, :]
                        nc.sync.dma_start(
                            out=dst.rearrange("m d h -> d m h"), in_=ost
                        )
    _split_multi_waits(nc)
    return nc


def _split_multi_waits(nc):
    """This walrus build accepts at most one semaphore wait per
    instruction; Tile emits up to ~2-4.  Hoist extra waits onto NoOp
    instructions inserted just before, on the same engine."""
    import concourse.mybir as mybir

    n_split = 0
    for fn in nc.m.functions:
        for bb in fn.blocks:
            insts = bb.instructions
            new = []
            for inst in insts:
                si = inst.sync_info
                waits = list(si.on_wait) if si is not None else []
                if len(waits) > 1:
                    for w in waits[:-1]:
                        nop = mybir.InstNoOp(
                            name=nc.get_next_instruction_name(), ins=[], outs=[]
                        )
                        nop.engine = inst.engine
                        nop.sync_info = mybir.SyncInfo(
                            on_update=[], on_wait=[w]
                        )
                        new.append(nop)
                        n_split += 1
                    si.on_wait = [waits[-1]]
                new.append(inst)
            if n_split:
                bb.instructions = new
    return n_split


def _get_nc():
    if "nc" not in _CACHE:
        _CACHE["nc"] = _build_nc()
    return _CACHE["nc"]


def _prep(G, W, b):
    # Block-diagonal W for phase 3: rows indexed (l, r) with r = m-within-
    # group, cols (r'', h); nonzero only when r == r''.
    MG = 4
    Wbd = np.zeros((K * K, P, MG * H), dtype=np.float32)
    for k in range(K):
        for j in range(K):
            blk = W[k * (K * L) + j * L:k * (K * L) + (j + 1) * L, :]  # [L, H]
            for l in range(L):
                for r in range(MG):
                    Wbd[k * K + j, l * MG + r, r * H:(r + 1) * H] = blk[l]
    Bb = np.tile(b[None, :], (P, MG)).astype(np.float32)
    return np.ascontiguousarray(Wbd), Bb


def kernel(X, G, W, b):
    import ml_dtypes
    from concourse.bass_utils import run_bass_kernel_spmd

    X = np.ascontiguousarray(X, dtype=np.float32)
    G = np.ascontiguousarray(G, dtype=np.float32)
    W = np.ascontiguousarray(W, dtype=np.float32)
    b = np.ascontiguousarray(b, dtype=np.float32)
    nc = _get_nc()
    Wr, Bb = _prep(G, W, b)
    Xb = X.astype(ml_dtypes.bfloat16)
    Gb = G.astype(ml_dtypes.bfloat16)
    in_maps = [
        {"X": Xb[i], "G": G, "GB": Gb, "WR": Wr, "BB": Bb} for i in range(B)
    ]
    res = run_bass_kernel_spmd(nc, in_maps, list(range(B)))
    out = np.stack([res.results[i]["OUT"] for i in range(B)], axis=0)
    return out
